# revision 3
# baseline (speedup 1.0000x reference)
"""Trainium2 distributed kernel for nn_AGNN_52673478918326.

Computes, on 8 NeuronCores (SPMD, data-parallel over the batch axis):
    lg = mean(labelgcn, axis=0)            # [63, 2048], batch=2048
    S1 = inverse_covariance_S(lg, adj)     # [63, 63]
    l1 = gac(lg, W1, b1, S1)               # [63, 1024]
    S2 = inverse_covariance_S(l1, adj)
    l2 = gac(l1, W2, b2, S2)               # [63, 300]
    returns (l2, S1, S2)

Sharding: each core reduces its 256-batch shard of labelgcn to a partial
sum [63*2048] laid out [128, 1008] in SBUF, AllReduce(add) across the 8
cores, then the (tiny) 63-node graph math runs replicated on every core.
"""

import os
import numpy as np

_REPO = "/opt/trn_rl_repo"

N = 63
IN = 2048
HID = 1024
OUT = 300
NCORES = 8
P = 128
FEAT = N * IN            # 129024
Q = FEAT // P            # 1008


def _ensure_path():
    import sys
    if _REPO not in sys.path:
        sys.path.insert(0, _REPO)


def build(bpc=256, g=8):
    """Build the SPMD Bass graph for one core (bpc batch elements/core,
    g batch elements per DMA chunk)."""
    _ensure_path()
    import concourse.bass as bass
    import concourse.bacc as bacc
    import concourse.mybir as mybir
    import concourse.tile as tile
    import concourse.masks as masks

    f32 = mybir.dt.float32
    AX = mybir.AxisListType.X
    OP = mybir.AluOpType
    ACT = mybir.ActivationFunctionType
    MEM = bass.MemorySpace

    assert bpc % g == 0
    nchunks = bpc // g
    KIN = IN // P            # 16
    KH = HID // P            # 8
    inv_batch = 1.0 / float(bpc * NCORES)

    nc = bacc.Bacc("TRN2", target_bir_lowering=False, debug=False,
                   num_devices=NCORES)

    x_ext = nc.dram_tensor("x", [bpc, N, IN], f32, kind="ExternalInput")
    adj_ext = nc.dram_tensor("adj", [N, N], f32, kind="ExternalInput")
    w1t_ext = nc.dram_tensor("w1t", [IN, HID], f32, kind="ExternalInput")
    b1_ext = nc.dram_tensor("b1", [1, HID], f32, kind="ExternalInput")
    w2t_ext = nc.dram_tensor("w2t", [HID, OUT], f32, kind="ExternalInput")
    b2_ext = nc.dram_tensor("b2", [1, OUT], f32, kind="ExternalInput")
    l2_ext = nc.dram_tensor("out_l2", [N, OUT], f32, kind="ExternalOutput")
    s1_ext = nc.dram_tensor("out_s1", [N, N], f32, kind="ExternalOutput")
    s2_ext = nc.dram_tensor("out_s2", [N, N], f32, kind="ExternalOutput")

    with tile.TileContext(nc) as tc:
        with (
            tc.tile_pool(name="dram", bufs=1, space="DRAM") as dpool,
            tc.tile_pool(name="const", bufs=1) as cpool,
            tc.tile_pool(name="wpool", bufs=1) as wpool,
            tc.tile_pool(name="gbuf", bufs=1) as gpool,
            tc.tile_pool(name="spool", bufs=1) as spool,
            tc.tile_pool(name="ps_tr", bufs=2, space=MEM.PSUM) as ps_tr,
            tc.tile_pool(name="ps_mm", bufs=2, space=MEM.PSUM) as ps_mm,
            tc.tile_pool(name="ps_sm", bufs=2, space=MEM.PSUM) as ps_sm,
        ):
            ar_in = dpool.tile([FEAT], f32)
            ar_out = dpool.tile([FEAT], f32)
            m1_dram = dpool.tile([N * HID], f32)
            m2_dram = dpool.tile([N * OUT], f32)

            ident = cpool.tile([P, P], f32)
            masks.make_identity(nc, ident[:])
            ones = cpool.tile([P, 1], f32)
            nc.gpsimd.memset(ones[:], 1.0)
            zcol = cpool.tile([P, 1], f32)
            nc.gpsimd.memset(zcol[:], 0.0)

            adj_sb = cpool.tile([N, N], f32)
            nc.sync.dma_start(adj_sb[:], adj_ext.ap())
            b1row = cpool.tile([1, HID], f32)
            nc.sync.dma_start(b1row[:], b1_ext.ap())
            b2row = cpool.tile([1, OUT], f32)
            nc.sync.dma_start(b2row[:], b2_ext.ap())
            b1b = cpool.tile([N, HID], f32)
            nc.gpsimd.partition_broadcast(b1b[:], b1row[:])
            b2b = cpool.tile([N, OUT], f32)
            nc.gpsimd.partition_broadcast(b2b[:], b2row[:])

            # ---------------- Phase A: batch-sum of the local shard -------
            acc = gpool.tile([P, Q], f32)
            nc.vector.memset(acc[:], 0.0)

            # x viewed so that each batch element's [63,2048] block becomes
            # a [128, 1008] SBUF tile (flat row-major reinterpretation).
            x_v = (x_ext.ap()
                   .rearrange("b n i -> b (n i)")
                   .rearrange("b (p q) -> p b q", p=P))

            with (
                tc.tile_pool(name="inp", bufs=2) as ipool,
                tc.tile_pool(name="red", bufs=2) as rpool,
            ):
                for c in range(nchunks):
                    t = ipool.tile([P, g * Q], f32, tag="in")
                    nc.sync.dma_start(
                        t[:].rearrange("p (b q) -> p b q", b=g),
                        x_v[:, c * g:(c + 1) * g, :],
                    )
                    r = rpool.tile([P, Q], f32, tag="r")
                    nc.vector.tensor_reduce(
                        r[:], t[:].rearrange("p (b q) -> p q b", b=g),
                        axis=AX, op=OP.add,
                    )
                    nc.vector.tensor_tensor(acc[:], acc[:], r[:], op=OP.add)

            # partial sums -> DRAM bounce, then AllReduce over the 8 cores.
            nc.sync.dma_start(ar_in[:].rearrange("(p q) -> p q", p=P), acc[:])

            # Weights stream on the sync ring right behind the bounce write,
            # i.e. during the AllReduce window when HBM is otherwise idle.
            w1s = wpool.tile([P, KIN * HID], f32)
            nc.sync.dma_start(
                w1s[:].rearrange("p (k h) -> p k h", k=KIN),
                w1t_ext.ap().rearrange("(k p) h -> p k h", p=P),
            )
            w2s = wpool.tile([P, KH * OUT], f32)
            nc.sync.dma_start(
                w2s[:].rearrange("p (k h) -> p k h", k=KH),
                w2t_ext.ap().rearrange("(k p) h -> p k h", p=P),
            )

            nc.gpsimd.collective_compute(
                "AllReduce",
                OP.add,
                replica_groups=[list(range(NCORES))],
                ins=[ar_in.opt()],
                outs=[ar_out.opt()],
            )

            # ---------------- Graph math (replicated on every core) -------
            lg = gpool.tile([N, IN], f32)
            nc.gpsimd.dma_start(lg[:], ar_out[:].rearrange("(n i) -> n i", n=N))
            nc.vector.tensor_scalar_mul(lg[:], lg[:], inv_batch)

            def pe_transpose(src_ap, rows, cols, dst_ap):
                """dst[cols, rows] = src[rows, cols].T via the PE array."""
                pt = ps_tr.tile([P, P], f32, tag="pt")
                nc.tensor.transpose(pt[0:cols, 0:rows], src_ap, ident[0:rows, 0:rows])
                nc.scalar.copy(dst_ap, pt[0:cols, 0:rows])

            # lgT tiles: lgT[:, k*63:(k+1)*63] = lg[:, k*128:(k+1)*128].T
            lgT = gpool.tile([P, KIN * N], f32)
            for k in range(KIN):
                pe_transpose(lg[:, k * P:(k + 1) * P], N, P,
                             lgT[:, k * N:(k + 1) * N])

            # x1 = lg @ lg.T  [63, 63]
            x1p = ps_mm.tile([N, N], f32, tag="mm")
            for k in range(KIN):
                nc.tensor.matmul(x1p[:], lgT[:, k * N:(k + 1) * N],
                                 lgT[:, k * N:(k + 1) * N],
                                 start=(k == 0), stop=(k == KIN - 1))
            x1 = spool.tile([N, N], f32, tag="x1")
            nc.scalar.copy(x1[:], x1p[:])

            def build_S(x_sb, s_tag):
                """S-construction from x=[63,63] (reference semantics:
                mn:=max(x), mx:=min(x), xn=sqrt((x-mn)/(mx-mn)),
                s=adj*xn/tr(xn), row-normalize with 0-rows kept)."""
                tg = lambda t: f"{s_tag}_{t}"
                nx = spool.tile([N, N], f32, tag=tg("nx"))
                nc.vector.tensor_scalar_mul(nx[:], x_sb[:], -1.0)
                colred = spool.tile([N, 2], f32, tag=tg("colred"))
                nc.vector.tensor_reduce(colred[:, 0:1], x_sb[:], axis=AX, op=OP.max)
                nc.vector.tensor_reduce(colred[:, 1:2], nx[:], axis=AX, op=OP.max)
                # Transpose each column separately so both land on partition 0
                # (compute engines cannot start at partition 1).
                cts = spool.tile([1, 2 * N], f32, tag=tg("cts"))
                for col in range(2):
                    ctp = ps_sm.tile([1, N], f32, tag="sm")
                    nc.tensor.transpose(ctp[:], colred[:, col:col + 1],
                                        ident[0:N, 0:N])
                    nc.scalar.copy(cts[:, col * N:(col + 1) * N], ctp[:])
                sc = spool.tile([1, 4], f32, tag=tg("sc"))
                # sc0 = mn (global max), sc1 = -mx (-global min)
                nc.vector.tensor_reduce(sc[:, 0:1], cts[:, 0:N], axis=AX, op=OP.max)
                nc.vector.tensor_reduce(sc[:, 1:2], cts[:, N:2 * N], axis=AX,
                                        op=OP.max)
                # sc2 = -(mn + (-mx)) = mx - mn ; sc3 = r = 1/(mx-mn)
                nc.vector.scalar_tensor_tensor(sc[:, 2:3], sc[:, 0:1], -1.0,
                                               sc[:, 1:2], op0=OP.mult,
                                               op1=OP.subtract)
                nc.vector.reciprocal(sc[:, 3:4], sc[:, 2:3])  # r = 1/(mx-mn)
                pk = spool.tile([1, 2], f32, tag=tg("pk"))
                nc.vector.tensor_copy(pk[:, 0:1], sc[:, 0:1])     # mn
                nc.vector.tensor_copy(pk[:, 1:2], sc[:, 3:4])     # r
                bc = spool.tile([N, 2], f32, tag=tg("bc"))
                nc.gpsimd.partition_broadcast(bc[:], pk[:])
                # xn = sqrt((x - mn) * r); (x-mn)<=0 exactly, r<0 -> arg>=0.
                tdiff = spool.tile([N, N], f32, tag=tg("tdiff"))
                nc.vector.tensor_scalar(tdiff[:], x_sb[:], bc[:, 0:1], None,
                                        op0=OP.subtract)
                xn = spool.tile([N, N], f32, tag=tg("xn"))
                nc.scalar.activation(xn[:], tdiff[:], ACT.Sqrt,
                                     bias=zcol[0:N, :], scale=bc[:, 1:2])
                # trace(xn)
                dtmp = spool.tile([N, N], f32, tag=tg("dtmp"))
                nc.vector.tensor_tensor(dtmp[:], xn[:], ident[0:N, 0:N], op=OP.mult)
                diag = spool.tile([N, 1], f32, tag=tg("diag"))
                nc.vector.reduce_sum(diag[:], dtmp[:], axis=AX)
                trp = ps_sm.tile([1, 1], f32, tag="sm")
                nc.tensor.matmul(trp[:], ones[0:N, 0:1], diag[:],
                                 start=True, stop=True)
                trs = spool.tile([1, 1], f32, tag=tg("trs"))
                nc.scalar.copy(trs[:], trp[:])
                trin = spool.tile([1, 1], f32, tag=tg("trin"))
                nc.vector.reciprocal(trin[:], trs[:])
                trb = spool.tile([N, 1], f32, tag=tg("trb"))
                nc.gpsimd.partition_broadcast(trb[:], trin[:])
                # s = (xn * (1/tr)) * adj
                smat = spool.tile([N, N], f32, tag=tg("smat"))
                nc.vector.scalar_tensor_tensor(smat[:], xn[:], trb[:, 0:1],
                                               adj_sb[:], op0=OP.mult, op1=OP.mult)
                row = spool.tile([N, 1], f32, tag=tg("row"))
                nc.vector.reduce_sum(row[:], smat[:], axis=AX)
                iz = spool.tile([N, 1], f32, tag=tg("iz"))
                nc.vector.tensor_scalar(iz[:], row[:], 0.0, None, op0=OP.is_equal)
                rowf = spool.tile([N, 1], f32, tag=tg("rowf"))
                nc.vector.tensor_tensor(rowf[:], row[:], iz[:], op=OP.add)
                rinv = spool.tile([N, 1], f32, tag=tg("rinv"))
                nc.vector.reciprocal(rinv[:], rowf[:])
                S = spool.tile([N, N], f32, tag=tg("S"))
                nc.vector.tensor_scalar_mul(S[:], smat[:], rinv[:, 0:1])
                return S

            S1 = build_S(x1, "s1")
            nc.gpsimd.dma_start(s1_ext.ap(), S1[:])

            def gac_mlp(lT, kcnt, w_sb, hdim, bias_b, mm_tag):
                """relu(L @ W.T + b): lT = [P, kcnt*63] transposed-L tiles,
                w_sb = [P, kcnt*hdim] transposed-W tiles -> [63, hdim]."""
                mp = ps_mm.tile([N, hdim], f32, tag="mm")
                for n0 in range(0, hdim, 512):
                    n1 = min(hdim, n0 + 512)
                    for k in range(kcnt):
                        nc.tensor.matmul(
                            mp[:, n0:n1],
                            lT[:, k * N:(k + 1) * N],
                            w_sb[:, k * hdim + n0:k * hdim + n1],
                            start=(k == 0), stop=(k == kcnt - 1))
                ms = spool.tile([N, hdim], f32, tag=f"{mm_tag}_s")
                nc.vector.tensor_tensor(ms[:], mp[:], bias_b[:], op=OP.add)
                nc.vector.tensor_scalar_max(ms[:], ms[:], 0.0)
                return ms

            def scramble(ms, hdim, dram_buf, s_tag):
                """F[n, h] = ms.flat[h*63 + n]  (reshape(hdim,63).T)."""
                nc.gpsimd.dma_start(
                    dram_buf[:].rearrange("(n h) -> n h", n=N), ms[:])
                ntiles = (hdim + P - 1) // P
                gt = spool.tile([P, ntiles * N], f32, tag=f"{s_tag}_gt")
                gv = dram_buf[:].rearrange("(h n) -> h n", n=N)
                F = spool.tile([N, hdim], f32, tag=f"{s_tag}_F")
                for t in range(ntiles):
                    cnt = min(P, hdim - t * P)
                    nc.gpsimd.dma_start(gt[0:cnt, t * N:(t + 1) * N],
                                        gv[t * P:t * P + cnt, :])
                    pe_transpose(gt[0:cnt, t * N:(t + 1) * N], cnt, N,
                                 F[:, t * P:t * P + cnt])
                return F

            # GAC 1
            m1s = gac_mlp(lgT, KIN, w1s, HID, b1b, "m1")
            F1 = scramble(m1s, HID, m1_dram, "scr1")
            s1t = spool.tile([N, N], f32, tag="s1t")
            pe_transpose(S1[:], N, N, s1t[:])
            l1p = ps_mm.tile([N, HID], f32, tag="mm")
            for n0 in range(0, HID, 512):
                nc.tensor.matmul(l1p[:, n0:n0 + 512], s1t[:],
                                 F1[:, n0:n0 + 512], start=True, stop=True)
            l1s = spool.tile([N, HID], f32, tag="l1s")
            nc.scalar.copy(l1s[:], l1p[:])

            # l1T tiles for the HID-contraction matmuls
            l1T = gpool.tile([P, KH * N], f32)
            for k in range(KH):
                pe_transpose(l1s[:, k * P:(k + 1) * P], N, P,
                             l1T[:, k * N:(k + 1) * N])

            # x2 = l1 @ l1.T
            x2p = ps_mm.tile([N, N], f32, tag="mm")
            for k in range(KH):
                nc.tensor.matmul(x2p[:], l1T[:, k * N:(k + 1) * N],
                                 l1T[:, k * N:(k + 1) * N],
                                 start=(k == 0), stop=(k == KH - 1))
            x2 = spool.tile([N, N], f32, tag="x2")
            nc.scalar.copy(x2[:], x2p[:])

            S2 = build_S(x2, "s2")
            nc.gpsimd.dma_start(s2_ext.ap(), S2[:])

            # GAC 2
            m2s = gac_mlp(l1T, KH, w2s, OUT, b2b, "m2")
            F2 = scramble(m2s, OUT, m2_dram, "scr2")
            s2t = spool.tile([N, N], f32, tag="s2t")
            pe_transpose(S2[:], N, N, s2t[:])
            l2p = ps_mm.tile([N, OUT], f32, tag="mm")
            nc.tensor.matmul(l2p[:], s2t[:], F2[:], start=True, stop=True)
            l2s = spool.tile([N, OUT], f32, tag="l2s")
            nc.scalar.copy(l2s[:], l2p[:])
            nc.gpsimd.dma_start(l2_ext.ap(), l2s[:])

    nc.compile()
    return nc


_NC_CACHE = {}


def _get_nc(bpc=256, g=8):
    key = (bpc, g)
    if key not in _NC_CACHE:
        _NC_CACHE[key] = build(bpc, g)
    return _NC_CACHE[key]


LAST_RESULT = None


def kernel(labelgcn, adj, W1, b1, W2, b2):
    _ensure_path()
    from concourse.bass_utils import run_bass_kernel_spmd

    labelgcn = np.ascontiguousarray(np.asarray(labelgcn, dtype=np.float32))
    adj = np.ascontiguousarray(np.asarray(adj, dtype=np.float32))
    w1t = np.ascontiguousarray(np.asarray(W1, dtype=np.float32).T)
    w2t = np.ascontiguousarray(np.asarray(W2, dtype=np.float32).T)
    b1r = np.ascontiguousarray(np.asarray(b1, dtype=np.float32).reshape(1, HID))
    b2r = np.ascontiguousarray(np.asarray(b2, dtype=np.float32).reshape(1, OUT))

    bpc = labelgcn.shape[0] // NCORES
    nc = _get_nc(bpc=bpc, g=8)

    in_maps = [
        {
            "x": labelgcn[c * bpc:(c + 1) * bpc],
            "adj": adj,
            "w1t": w1t,
            "b1": b1r,
            "w2t": w2t,
            "b2": b2r,
        }
        for c in range(NCORES)
    ]
    trace = bool(int(os.environ.get("KERNEL_TRACE", "0")))
    res = run_bass_kernel_spmd(nc, in_maps, core_ids=list(range(NCORES)),
                               trace=trace)
    global LAST_RESULT
    LAST_RESULT = res
    r0 = res.results[0]
    return (np.asarray(r0["out_l2"]), np.asarray(r0["out_s1"]),
            np.asarray(r0["out_s2"]))
